# revision 12
# baseline (speedup 1.0000x reference)
"""Single-head causal attention (B=8, T=2048, C=1024, H=64) on 8 NeuronCores.

Data-parallel over batch: core b computes attention for x[b].

v4 design:
  * Host stages x as bf16 window-contiguous [p][w][c][t]; weights blob
    + 8 half-window HWDGE DMAs on the sync ring (issue cost ~0.65us
    per dma_start makes fewer/bigger transfers win).
  * Chain A = [Wk|Wv], chain B = [Wq|Wk]: the k row-64..127 duplicate
    (for even/odd key-tile PE-array row pairing) comes free from chain
    B's upper half; only the q duplicate needs a shift-matmul
    (lhsT = [0|I64]) + DVE cast.  No DMA dups, no SWDGE.
  * Windows 2,3 run chunk-major (one LDWEIGHTS serves both windows'
    matmuls) to cut exposed weight-load time.
  * 40 warmup matmuls bridge the idle gap until window 0 lands so the
    HAM clock-gate stays at 8/8 when real work starts; exp table
    loaded at t=0 by a garbage ACTIVATE.
  * S pairs -> one 2-bank f32 PSUM tile, one ACTIVATE per pair
    (block 0's first pair is split even/odd so exp starts before the
    q-dup completes).
  * Global pipeline: exp(g) -> masks -> S(g+2) -> PV(g) -> 4 proj
    pops; tail finish pipelined (split ot copies, split out DMA).
"""

import numpy as np
import ml_dtypes

import concourse.bass as bass
import concourse.bacc as bacc
import concourse.mybir as mybir
import concourse.tile as tile
from concourse.bass_utils import run_bass_kernel_spmd


B = 8
T, C, H = 2048, 1024, 64
P = 128
NCH = C // P     # 8 C-chunks
NT = T // P      # 16 T-tiles
QT = 512         # query-block width
NQ = T // QT     # 4 query blocks / x windows
H1 = H + 1
f32 = mybir.dt.float32
bf16 = mybir.dt.bfloat16
i16 = mybir.dt.int16
EXP = mybir.ActivationFunctionType.Exp
BF16NP = np.dtype(ml_dtypes.bfloat16)

# exp via bf16-bit trick on DVE for some pairs: scores are scaled by
# A = 128*log2(e) (folded into Wq, on top of 1/sqrt(H)); ACT pairs undo
# it with the activation's free scale; DVE pairs add the exponent bias
# and convert to int16, whose bits ARE bf16 2^(x) (Schraudolph).
ASCHR = 128.0 * np.log2(np.e)
ACT_SCALE = float(np.log(2.0) / 128.0)
SCHR_BIAS = 16256.0 - 4.5  # 127<<7 minus the linear-interp centering C

# weights blob layout (columns)
WKV0 = 0                  # [P, NCH*P]  [Wk|Wv] chunk-interleaved
WQK0 = NCH * P            # [P, NCH*P]  [Wq/sqrt(H)|Wk] chunk-interleaved
WID0 = WQK0 + NCH * P     # [P, P]      identity
WSH0 = WID0 + P           # [P, P]      shift identity: rows 0-63, cols 64-127 = I64
WCOLS = WSH0 + P


def width(i, j):
    d = j - 4 * i
    return QT - d * P if d > 0 else QT


PAIRS = [(i, jj) for i in range(NQ) for jj in range(2 * (i + 1))]


def build_nc() -> bass.Bass:
    nc = bacc.Bacc("TRN2", target_bir_lowering=False, debug=False)
    X = nc.dram_tensor("X", [P, NQ * NCH * QT], bf16, kind="ExternalInput")
    WB = nc.dram_tensor("WB", [P, WCOLS], bf16, kind="ExternalInput")
    out = nc.dram_tensor("out", [T, H], f32, kind="ExternalOutput")

    with tile.TileContext(nc) as tc:
        with (
            tc.tile_pool(name="const", bufs=1) as constp,
            tc.tile_pool(name="w", bufs=1) as wp,
            tc.tile_pool(name="xt", bufs=4) as xtp,
            tc.tile_pool(name="qkv", bufs=1) as qkvp,
            tc.tile_pool(name="pt", bufs=4) as ptp,
            tc.tile_pool(name="fin", bufs=4) as finp,
            tc.tile_pool(name="ps", bufs=2, space="PSUM") as psp,      # chains
            tc.tile_pool(name="sp", bufs=2, space="PSUM") as spairp,   # S pairs
            tc.tile_pool(name="acc", bufs=1, space="PSUM") as accp,    # po
            tc.tile_pool(name="misc", bufs=1, space="PSUM") as miscp,  # pv + pob
        ):
            # --- sync HWDGE ring: weights blob, then 8 half-window DMAs ---
            wb = wp.tile([P, WCOLS], bf16, tag="wb")
            nc.scalar.dma_start(out=wb, in_=WB[:, :])
            wkv_r = wb[:, WKV0 : WKV0 + NCH * P]
            wqk_r = wb[:, WQK0 : WQK0 + NCH * P]
            ident = wb[:, WID0 : WID0 + P]
            ishift = wb[:, WSH0 : WSH0 + P]

            Xv = X.rearrange("p (w c t) -> p w c t", c=NCH, t=QT)
            xvs = []
            for w in range(NQ):
                xtw = xtp.tile([P, NCH * QT], bf16, tag="xtw", name=f"xtw{w}")
                xv = xtw.rearrange("p (c t) -> p c t", t=QT)
                nc.sync.dma_start(out=xv[:, 0:4, :], in_=Xv[:, w, 0:4, :])
                nc.sync.dma_start(out=xv[:, 4:8, :], in_=Xv[:, w, 4:8, :])
                xvs.append(xv)

            # --- persistent SBUF tensors ---
            qq = qkvp.tile([P, T], bf16, tag="qq")   # q rows 0-63, q-dup rows 64-127
            kp = qkvp.tile([P, T // 2], bf16, tag="kp")  # pair jj: even tile @0-63, odd @64-127
            vt = qkvp.tile([P, T], bf16, tag="vt")       # vT at partitions 64-127
            vsb = qkvp.tile([P, NT * H1], bf16, tag="vsb")  # v natural + ones col
            vsb_v = vsb.rearrange("p (t w) -> p t w", w=H1)
            ones = constp.tile([P, NT], f32, tag="ones")
            nc.vector.memset(ones, 1.0)
            nc.vector.tensor_copy(vsb_v[:, :, H:H1], ones.unsqueeze(2))
            osb = finp.tile([P, NT * H], f32, tag="osb", bufs=1)

            garbage = constp.tile([P, P], bf16, tag="garbage")
            nc.vector.memset(garbage, 1.0)

            warm_act = constp.tile([P, 8], bf16, tag="warm_act")
            nc.scalar.activation(warm_act, garbage[:, 0:8], EXP)

            misc = miscp.tile([P, 512], f32, tag="misc")
            pv_view = misc.rearrange("p (k h) -> p k h", h=H)  # k=0..3 used

            # --- PE warmup into the first S-pair buffer (HAM 8/8 until w0) ---
            warm = spairp.tile([P, 2 * QT], f32, tag="spair", name="warm")
            for _ in range(64):
                nc.tensor.matmul(warm[:, 0:P], garbage, garbage,
                                 start=True, stop=True)

            def project_ops(w):
                """closures: chains + casts + q-dup + v-transpose, window w."""
                xv = xvs[w]
                ka = psp.tile([P, QT], f32, tag="chain", name=f"ka{w}")
                qk = psp.tile([P, QT], f32, tag="chain", name=f"qk{w}")
                kcols = slice(w * QT, (w + 1) * QT)
                qcols = slice(w * QT, (w + 1) * QT)
                pcols = slice(w * 2 * P, (w + 1) * 2 * P)
                ops = []
                for c in range(NCH):
                    ops.append(lambda c=c: nc.tensor.matmul(
                        ka, wkv_r[:, c * P : (c + 1) * P], xv[:, c, :],
                        start=(c == 0), stop=(c == NCH - 1)))
                    ops.append(lambda c=c: nc.tensor.matmul(
                        qk, wqk_r[:, c * P : (c + 1) * P], xv[:, c, :],
                        start=(c == 0), stop=(c == NCH - 1)))
                # q + k first: next block's S pairs need them
                ops.append(lambda: nc.vector.tensor_copy(qq[0:H, qcols], qk[0:H, :]))
                # even k tiles (chain A rows 0-63, tiles 0,2 of window) -> kp rows 0-63
                ops.append(lambda: nc.vector.tensor_copy(
                    kp[0:H, pcols].rearrange("p (a b) -> p a b", b=P),
                    ka[0:H, :].rearrange("p (a b) -> p a b", b=P)[:, 0::2, :]))
                # odd k tiles (chain B rows 64-127, tiles 1,3) -> kp rows 64-127
                ops.append(lambda: nc.vector.tensor_copy(
                    kp[H:P, pcols].rearrange("p (a b) -> p a b", b=P),
                    qk[H:P, :].rearrange("p (a b) -> p a b", b=P)[:, 1::2, :]))
                # q-dup via shift-matmul into qk's bank (rows 64-127)
                ops.append(lambda: nc.tensor.matmul(
                    qk, ishift[0:H, :], qq[0:H, qcols], start=True, stop=True))
                ops.append(lambda: nc.vector.tensor_copy(qq[H:P, qcols], qk[H:P, :]))
                ops.append(lambda: nc.vector.tensor_copy(vt[H:P, kcols], ka[H:P, :]))
                for k in range(4):
                    ops.append(lambda k=k: nc.tensor.matmul(
                        pv_view[:, k, :],
                        vt[H:P, (4 * w + k) * P : (4 * w + k + 1) * P],
                        ident[H:P, H:P], start=True, stop=True))
                ops.append(lambda: nc.vector.tensor_copy(
                    vsb_v[:, 4 * w : 4 * w + 4, 0:H], pv_view[:, 0:4, :]))
                return ops

            def s_even(i, jj, ps, skip_ldw=False):
                w = width(i, 2 * jj)
                qoff = i * QT + (QT - w)
                mm = nc.tensor.matmul(
                    ps[:, 0:w], kp[0:H, jj * P : (jj + 1) * P],
                    qq[0:H, qoff : qoff + w], start=True, stop=True)
                if skip_ldw:
                    mm.ins.ldweights = False

            def s_odd(i, jj, ps, skip_ldw=False):
                w = width(i, 2 * jj + 1)
                qoff = i * QT + (QT - w)
                mm = nc.tensor.matmul(
                    ps[:, QT : QT + w], kp[H:P, jj * P : (jj + 1) * P],
                    qq[H:P, qoff : qoff + w], start=True, stop=True)
                if skip_ldw:
                    mm.ins.ldweights = False

            def s_pair(i, jj):
                ps = spairp.tile([P, 2 * QT], f32, tag="spair",
                                 name=f"s{i}_{jj}")
                nc.tensor.ldweights(kp[:, jj * P : (jj + 1) * P])
                s_even(i, jj, ps, skip_ldw=True)
                s_odd(i, jj, ps, skip_ldw=True)
                return ps

            def exp_pair(i, jj, ps):
                wmax = width(i, 2 * jj)
                srcv = ps.rearrange("p (s t) -> p s t", s=2)[:, :, 0:wmax]
                if i >= 2 and jj % 2 == 1 and jj != 2 * i + 1:
                    pt = ptp.tile([P, 2 * QT], i16, tag="pt",
                                  name=f"pt{i}_{jj}")
                    dst = pt.rearrange("p (s t) -> p s t", s=2)[:, :, 0:wmax]
                    nc.vector.tensor_scalar_add(dst, srcv, SCHR_BIAS)
                else:
                    pt = ptp.tile([P, 2 * QT], bf16, tag="pt",
                                  name=f"pt{i}_{jj}")
                    dst = pt.rearrange("p (s t) -> p s t", s=2)[:, :, 0:wmax]
                    nc.scalar.activation(dst, srcv, EXP, scale=ACT_SCALE)
                return pt

            def pt_slice(pt, u, w):
                v = pt.rearrange("p (s t) -> p s t", s=2)[:, u, 0:w]
                if v.dtype != bf16:
                    v = v.bitcast(bf16)
                return v

            def masks(i, jj, pt):
                for u in (0, 1):
                    j = 2 * jj + u
                    if j >= 4 * i:
                        w = width(i, j)
                        v = pt_slice(pt, u, w)
                        nc.gpsimd.affine_select(
                            out=v, in_=v,
                            pattern=[[1, w]],
                            compare_op=mybir.AluOpType.is_ge, fill=0.0,
                            base=0, channel_multiplier=-1)

            def pv_pair(i, jj, pt, po):
                nj = 4 * (i + 1)
                for u in (0, 1):
                    j = 2 * jj + u
                    w = width(i, j)
                    nc.tensor.matmul(
                        po[0:H1, QT - w : QT],
                        vsb[:, j * H1 : (j + 1) * H1],
                        pt_slice(pt, u, w),
                        start=(j == 0), stop=(j == nj - 1))

            def finish_block(i, po):
                ot = finp.tile([H1, QT], bf16, tag="ot")
                last = i == NQ - 1
                for half in range(2):
                    cols = slice(half * 256, (half + 1) * 256)
                    if last and half == 1:
                        nc.vector.tensor_copy(ot[:, cols], po[0:H1, cols])
                    else:
                        nc.scalar.copy(ot[:, cols], po[0:H1, cols])
                    for b in (2 * half, 2 * half + 1):
                        t = 4 * i + b
                        if last:
                            # proj is done: reuse the dead v-transpose region
                            # so all 4 transposes get their own slot
                            pob = misc[:, b * 96 : b * 96 + H1]
                        else:
                            pob = misc[:, 256 + (b % 2) * 96 : 256 + (b % 2) * 96 + H1]
                        nc.tensor.matmul(pob, ot[:, b * P : (b + 1) * P],
                                         ident[0:H1, 0:H1], start=True, stop=True)
                        rcp = finp.tile([P, 1], f32, tag="rcp")
                        nc.vector.reciprocal(rcp, pob[:, H:H1])
                        nc.vector.tensor_scalar_mul(
                            osb[:, t * H : (t + 1) * H], pob[:, 0:H], rcp)
                    nc.sync.dma_start(
                        out=out.rearrange("(t p) h -> p t h", p=P)[
                            :, 4 * i + 2 * half : 4 * i + 2 * half + 2, :],
                        in_=osb.rearrange("p (t h) -> p t h", h=H)[
                            :, 4 * i + 2 * half : 4 * i + 2 * half + 2, :])

            # ---- prologue: w0, first S pair split even/odd, w1 ----
            for op in project_ops(0):
                op()
            ps0 = spairp.tile([P, 2 * QT], f32, tag="spair", name="s0_0")
            s_even(0, 0, ps0)
            pt0 = ptp.tile([P, 2 * QT], bf16, tag="pt", name="pt0_0")
            nc.scalar.activation(pt0[:, 0:QT], ps0[:, 0:QT], EXP, scale=ACT_SCALE)
            s_odd(0, 0, ps0)
            sp_live = {1: s_pair(0, 1)}
            for op in project_ops(1):
                op()

            pend = {2: project_ops(2), 3: project_ops(3)}

            def pop_proj(k):
                for _ in range(k):
                    if pend[2]:
                        pend[2].pop(0)()
                    elif pend[3]:
                        pend[3].pop(0)()

            # ---- global attention pipeline ----
            po = None
            ptv0 = pt0.rearrange("p (s t) -> p s t", s=2)
            for g, (i, jj) in enumerate(PAIRS):
                if jj == 0:
                    if i == NQ - 1:
                        # proj done by block 3: reuse a free chain bank so the
                        # first PV needn't wait for block 2's po evacuation
                        po = psp.tile([P, QT], f32, tag="chain", name=f"po{i}")
                    else:
                        po = accp.tile([P, QT], f32, tag="po", name=f"po{i}")
                if g == 0:
                    # odd half of the split first pair
                    w1 = width(0, 1)
                    nc.scalar.activation(ptv0[:, 1, 0:w1],
                                         ps0.rearrange("p (s t) -> p s t", s=2)[:, 1, 0:w1],
                                         EXP, scale=ACT_SCALE)
                    pt = pt0
                else:
                    pt = exp_pair(i, jj, sp_live.pop(g))
                masks(i, jj, pt)
                if g + 2 < len(PAIRS):
                    ni = PAIRS[g + 2][0]
                    if ni >= 2:
                        while pend[ni]:
                            pend[ni].pop(0)()
                    sp_live[g + 2] = s_pair(*PAIRS[g + 2])
                pv_pair(i, jj, pt, po)
                pop_proj(4)
                if jj == 2 * (i + 1) - 1:
                    finish_block(i, po)

    nc.compile()
    return nc


_NC_CACHE = None


def _get_nc():
    global _NC_CACHE
    if _NC_CACHE is None:
        _NC_CACHE = build_nc()
    return _NC_CACHE


def run(in_maps, trace=False, **kw):
    nc = _get_nc()
    return run_bass_kernel_spmd(nc, in_maps, core_ids=list(range(B)),
                                trace=trace, **kw)


def _pack_weights(Wq, Wk, Wv):
    wb = np.zeros((P, WCOLS), dtype=np.float32)
    scale = np.float32(ASCHR / np.sqrt(H))
    for c in range(NCH):
        rows = slice(c * P, (c + 1) * P)
        wb[:, WKV0 + c * P : WKV0 + c * P + H] = Wk[rows, :]
        wb[:, WKV0 + c * P + H : WKV0 + (c + 1) * P] = Wv[rows, :]
        wb[:, WQK0 + c * P : WQK0 + c * P + H] = Wq[rows, :] * scale
        wb[:, WQK0 + c * P + H : WQK0 + (c + 1) * P] = Wk[rows, :]
    wb[:, WID0 : WID0 + P] = np.eye(P, dtype=np.float32)
    wb[0:H, WSH0 + H : WSH0 + P] = np.eye(H, dtype=np.float32)
    return wb.astype(BF16NP)


def make_in_maps(x, Wq, Wk, Wv):
    x = np.asarray(x, dtype=np.float32)
    Wq = np.asarray(Wq, dtype=np.float32)
    Wk = np.asarray(Wk, dtype=np.float32)
    Wv = np.asarray(Wv, dtype=np.float32)
    wb = _pack_weights(Wq, Wk, Wv)
    ins = []
    for b in range(B):
        A = x[b].reshape(NQ, QT, NCH, P).transpose(3, 0, 2, 1)
        ins.append({
            "X": np.ascontiguousarray(A).astype(BF16NP).reshape(P, NQ * NCH * QT),
            "WB": wb,
        })
    return ins


def kernel(x, Wq, Wk, Wv):
    res = run(make_in_maps(x, Wq, Wk, Wv))
    return np.stack([res.results[b]["out"] for b in range(B)], axis=0)


# revision 13
# speedup vs baseline: 1.0041x; 1.0041x over previous
"""Single-head causal attention (B=8, T=2048, C=1024, H=64) on 8 NeuronCores.

Data-parallel over batch: core b computes attention for x[b].

v4 design:
  * Host stages x as bf16 window-contiguous [p][w][c][t]; weights blob
    + 8 half-window HWDGE DMAs on the sync ring (issue cost ~0.65us
    per dma_start makes fewer/bigger transfers win).
  * Chain A = [Wk|Wv], chain B = [Wq|Wk]: the k row-64..127 duplicate
    (for even/odd key-tile PE-array row pairing) comes free from chain
    B's upper half; only the q duplicate needs a shift-matmul
    (lhsT = [0|I64]) + DVE cast.  No DMA dups, no SWDGE.
  * Windows 2,3 run chunk-major (one LDWEIGHTS serves both windows'
    matmuls) to cut exposed weight-load time.
  * 40 warmup matmuls bridge the idle gap until window 0 lands so the
    HAM clock-gate stays at 8/8 when real work starts; exp table
    loaded at t=0 by a garbage ACTIVATE.
  * S pairs -> one 2-bank f32 PSUM tile, one ACTIVATE per pair
    (block 0's first pair is split even/odd so exp starts before the
    q-dup completes).
  * Global pipeline: exp(g) -> masks -> S(g+2) -> PV(g) -> 4 proj
    pops; tail finish pipelined (split ot copies, split out DMA).
"""

import numpy as np
import ml_dtypes

import concourse.bass as bass
import concourse.bacc as bacc
import concourse.mybir as mybir
import concourse.tile as tile
from concourse.bass_utils import run_bass_kernel_spmd


B = 8
T, C, H = 2048, 1024, 64
P = 128
NCH = C // P     # 8 C-chunks
NT = T // P      # 16 T-tiles
QT = 512         # query-block width
NQ = T // QT     # 4 query blocks / x windows
H1 = H + 1
f32 = mybir.dt.float32
bf16 = mybir.dt.bfloat16
i16 = mybir.dt.int16
EXP = mybir.ActivationFunctionType.Exp
BF16NP = np.dtype(ml_dtypes.bfloat16)

# exp via bf16-bit trick on DVE for some pairs: scores are scaled by
# A = 128*log2(e) (folded into Wq, on top of 1/sqrt(H)); ACT pairs undo
# it with the activation's free scale; DVE pairs add the exponent bias
# and convert to int16, whose bits ARE bf16 2^(x) (Schraudolph).
ASCHR = 128.0 * np.log2(np.e)
ACT_SCALE = float(np.log(2.0) / 128.0)
SCHR_BIAS = 16256.0 - 4.5  # 127<<7 minus the linear-interp centering C

# weights blob layout (columns)
WKV0 = 0                  # [P, NCH*P]  [Wk|Wv] chunk-interleaved
WQK0 = NCH * P            # [P, NCH*P]  [Wq/sqrt(H)|Wk] chunk-interleaved
WID0 = WQK0 + NCH * P     # [P, P]      identity
WSH0 = WID0 + P           # [P, P]      shift identity: rows 0-63, cols 64-127 = I64
WCOLS = WSH0 + P


def width(i, j):
    d = j - 4 * i
    return QT - d * P if d > 0 else QT


PAIRS = [(i, jj) for i in range(NQ) for jj in range(2 * (i + 1))]


def build_nc() -> bass.Bass:
    nc = bacc.Bacc("TRN2", target_bir_lowering=False, debug=False)
    X = nc.dram_tensor("X", [P, NQ * NCH * QT], bf16, kind="ExternalInput")
    WB = nc.dram_tensor("WB", [P, WCOLS], bf16, kind="ExternalInput")
    out = nc.dram_tensor("out", [T, H], f32, kind="ExternalOutput")

    with tile.TileContext(nc) as tc:
        with (
            tc.tile_pool(name="const", bufs=1) as constp,
            tc.tile_pool(name="w", bufs=1) as wp,
            tc.tile_pool(name="xt", bufs=4) as xtp,
            tc.tile_pool(name="qkv", bufs=1) as qkvp,
            tc.tile_pool(name="pt", bufs=4) as ptp,
            tc.tile_pool(name="fin", bufs=4) as finp,
            tc.tile_pool(name="ps", bufs=2, space="PSUM") as psp,      # chains
            tc.tile_pool(name="sp", bufs=2, space="PSUM") as spairp,   # S pairs
            tc.tile_pool(name="acc", bufs=1, space="PSUM") as accp,    # po
            tc.tile_pool(name="misc", bufs=1, space="PSUM") as miscp,  # pv + pob
        ):
            # --- sync HWDGE ring: weights blob, then 8 half-window DMAs ---
            wb = wp.tile([P, WCOLS], bf16, tag="wb")
            nc.scalar.dma_start(out=wb, in_=WB[:, :])
            wkv_r = wb[:, WKV0 : WKV0 + NCH * P]
            wqk_r = wb[:, WQK0 : WQK0 + NCH * P]
            ident = wb[:, WID0 : WID0 + P]
            ishift = wb[:, WSH0 : WSH0 + P]

            Xv = X.rearrange("p (w c t) -> p w c t", c=NCH, t=QT)
            xvs = []
            for w in range(NQ):
                xtw = xtp.tile([P, NCH * QT], bf16, tag="xtw", name=f"xtw{w}")
                xv = xtw.rearrange("p (c t) -> p c t", t=QT)
                nc.sync.dma_start(out=xv[:, 0:4, :], in_=Xv[:, w, 0:4, :])
                nc.sync.dma_start(out=xv[:, 4:8, :], in_=Xv[:, w, 4:8, :])
                xvs.append(xv)

            # --- persistent SBUF tensors ---
            qq = qkvp.tile([P, T], bf16, tag="qq")   # q rows 0-63, q-dup rows 64-127
            kp = qkvp.tile([P, T // 2], bf16, tag="kp")  # pair jj: even tile @0-63, odd @64-127
            vt = qkvp.tile([P, T], bf16, tag="vt")       # vT at partitions 64-127
            vsb = qkvp.tile([P, NT * H1], bf16, tag="vsb")  # v natural + ones col
            vsb_v = vsb.rearrange("p (t w) -> p t w", w=H1)
            ones = constp.tile([P, NT], f32, tag="ones")
            nc.vector.memset(ones, 1.0)
            nc.vector.tensor_copy(vsb_v[:, :, H:H1], ones.unsqueeze(2))
            osb = finp.tile([P, NT * H], f32, tag="osb", bufs=1)

            garbage = constp.tile([P, P], bf16, tag="garbage")
            nc.vector.memset(garbage, 1.0)

            warm_act = constp.tile([P, 8], bf16, tag="warm_act")
            nc.scalar.activation(warm_act, garbage[:, 0:8], EXP)

            misc = miscp.tile([P, 512], f32, tag="misc")
            pv_view = misc.rearrange("p (k h) -> p k h", h=H)  # k=0..3 used

            # --- PE warmup into the first S-pair buffer (HAM 8/8 until w0) ---
            warm = spairp.tile([P, 2 * QT], f32, tag="spair", name="warm")
            for _ in range(64):
                nc.tensor.matmul(warm[:, 0:P], garbage, garbage,
                                 start=True, stop=True)

            def project_ops(w):
                """closures: chains + casts + q-dup + v-transpose, window w."""
                xv = xvs[w]
                ka = psp.tile([P, QT], f32, tag="chain", name=f"ka{w}")
                qk = psp.tile([P, QT], f32, tag="chain", name=f"qk{w}")
                kcols = slice(w * QT, (w + 1) * QT)
                qcols = slice(w * QT, (w + 1) * QT)
                pcols = slice(w * 2 * P, (w + 1) * 2 * P)
                ops = []
                for c in range(NCH):
                    ops.append(lambda c=c: nc.tensor.matmul(
                        ka, wkv_r[:, c * P : (c + 1) * P], xv[:, c, :],
                        start=(c == 0), stop=(c == NCH - 1)))
                    ops.append(lambda c=c: nc.tensor.matmul(
                        qk, wqk_r[:, c * P : (c + 1) * P], xv[:, c, :],
                        start=(c == 0), stop=(c == NCH - 1)))
                # q + k first: next block's S pairs need them
                ops.append(lambda: nc.vector.tensor_copy(qq[0:H, qcols], qk[0:H, :]))
                # even k tiles (chain A rows 0-63, tiles 0,2 of window) -> kp rows 0-63
                ops.append(lambda: nc.vector.tensor_copy(
                    kp[0:H, pcols].rearrange("p (a b) -> p a b", b=P),
                    ka[0:H, :].rearrange("p (a b) -> p a b", b=P)[:, 0::2, :]))
                # odd k tiles (chain B rows 64-127, tiles 1,3) -> kp rows 64-127
                ops.append(lambda: nc.vector.tensor_copy(
                    kp[H:P, pcols].rearrange("p (a b) -> p a b", b=P),
                    qk[H:P, :].rearrange("p (a b) -> p a b", b=P)[:, 1::2, :]))
                # q-dup via shift-matmul into qk's bank (rows 64-127)
                ops.append(lambda: nc.tensor.matmul(
                    qk, ishift[0:H, :], qq[0:H, qcols], start=True, stop=True))
                ops.append(lambda: nc.vector.tensor_copy(qq[H:P, qcols], qk[H:P, :]))
                ops.append(lambda: nc.vector.tensor_copy(vt[H:P, kcols], ka[H:P, :]))
                for k in range(4):
                    ops.append(lambda k=k: nc.tensor.matmul(
                        pv_view[:, k, :],
                        vt[H:P, (4 * w + k) * P : (4 * w + k + 1) * P],
                        ident[H:P, H:P], start=True, stop=True))
                ops.append(lambda: nc.vector.tensor_copy(
                    vsb_v[:, 4 * w : 4 * w + 4, 0:H], pv_view[:, 0:4, :]))
                return ops

            def s_even(i, jj, ps, skip_ldw=False):
                w = width(i, 2 * jj)
                qoff = i * QT + (QT - w)
                mm = nc.tensor.matmul(
                    ps[:, 0:w], kp[0:H, jj * P : (jj + 1) * P],
                    qq[0:H, qoff : qoff + w], start=True, stop=True)
                if skip_ldw:
                    mm.ins.ldweights = False

            def s_odd(i, jj, ps, skip_ldw=False):
                w = width(i, 2 * jj + 1)
                qoff = i * QT + (QT - w)
                mm = nc.tensor.matmul(
                    ps[:, QT : QT + w], kp[H:P, jj * P : (jj + 1) * P],
                    qq[H:P, qoff : qoff + w], start=True, stop=True)
                if skip_ldw:
                    mm.ins.ldweights = False

            def s_pair(i, jj):
                ps = spairp.tile([P, 2 * QT], f32, tag="spair",
                                 name=f"s{i}_{jj}")
                nc.tensor.ldweights(kp[:, jj * P : (jj + 1) * P])
                s_even(i, jj, ps, skip_ldw=True)
                s_odd(i, jj, ps, skip_ldw=True)
                return ps

            def exp_pair(i, jj, ps):
                wmax = width(i, 2 * jj)
                srcv = ps.rearrange("p (s t) -> p s t", s=2)[:, :, 0:wmax]
                if i >= 2 and jj % 2 == 1 and jj != 2 * i + 1:
                    pt = ptp.tile([P, 2 * QT], i16, tag="pt",
                                  name=f"pt{i}_{jj}")
                    dst = pt.rearrange("p (s t) -> p s t", s=2)[:, :, 0:wmax]
                    nc.vector.tensor_scalar_add(dst, srcv, SCHR_BIAS)
                else:
                    pt = ptp.tile([P, 2 * QT], bf16, tag="pt",
                                  name=f"pt{i}_{jj}")
                    dst = pt.rearrange("p (s t) -> p s t", s=2)[:, :, 0:wmax]
                    nc.scalar.activation(dst, srcv, EXP, scale=ACT_SCALE)
                return pt

            def pt_slice(pt, u, w):
                v = pt.rearrange("p (s t) -> p s t", s=2)[:, u, 0:w]
                if v.dtype != bf16:
                    v = v.bitcast(bf16)
                return v

            def masks(i, jj, pt):
                for u in (0, 1):
                    j = 2 * jj + u
                    if j >= 4 * i:
                        w = width(i, j)
                        v = pt_slice(pt, u, w)
                        nc.gpsimd.affine_select(
                            out=v, in_=v,
                            pattern=[[1, w]],
                            compare_op=mybir.AluOpType.is_ge, fill=0.0,
                            base=0, channel_multiplier=-1)

            def pv_pair(i, jj, pt, po):
                nj = 4 * (i + 1)
                for u in (0, 1):
                    j = 2 * jj + u
                    w = width(i, j)
                    nc.tensor.matmul(
                        po[0:H1, QT - w : QT],
                        vsb[:, j * H1 : (j + 1) * H1],
                        pt_slice(pt, u, w),
                        start=(j == 0), stop=(j == nj - 1))

            def finish_block(i, po):
                ot = finp.tile([H1, QT], bf16, tag="ot")
                last = i == NQ - 1
                for half in range(2):
                    cols = slice(half * 256, (half + 1) * 256)
                    nc.scalar.copy(ot[:, cols], po[0:H1, cols])
                    for b in (2 * half, 2 * half + 1):
                        t = 4 * i + b
                        if last:
                            # proj is done: reuse the dead v-transpose region
                            # so all 4 transposes get their own slot
                            pob = misc[:, b * 96 : b * 96 + H1]
                        else:
                            pob = misc[:, 256 + (b % 2) * 96 : 256 + (b % 2) * 96 + H1]
                        nc.tensor.matmul(pob, ot[:, b * P : (b + 1) * P],
                                         ident[0:H1, 0:H1], start=True, stop=True)
                        rcp = finp.tile([P, 1], f32, tag="rcp")
                        nc.vector.reciprocal(rcp, pob[:, H:H1])
                        nc.vector.tensor_scalar_mul(
                            osb[:, t * H : (t + 1) * H], pob[:, 0:H], rcp)
                    nc.sync.dma_start(
                        out=out.rearrange("(t p) h -> p t h", p=P)[
                            :, 4 * i + 2 * half : 4 * i + 2 * half + 2, :],
                        in_=osb.rearrange("p (t h) -> p t h", h=H)[
                            :, 4 * i + 2 * half : 4 * i + 2 * half + 2, :])

            # ---- prologue: w0, first S pair split even/odd, w1 ----
            for op in project_ops(0):
                op()
            ps0 = spairp.tile([P, 2 * QT], f32, tag="spair", name="s0_0")
            s_even(0, 0, ps0)
            pt0 = ptp.tile([P, 2 * QT], bf16, tag="pt", name="pt0_0")
            nc.scalar.activation(pt0[:, 0:QT], ps0[:, 0:QT], EXP, scale=ACT_SCALE)
            s_odd(0, 0, ps0)
            sp_live = {1: s_pair(0, 1)}
            for op in project_ops(1):
                op()

            pend = {2: project_ops(2), 3: project_ops(3)}

            def pop_proj(k):
                for _ in range(k):
                    if pend[2]:
                        pend[2].pop(0)()
                    elif pend[3]:
                        pend[3].pop(0)()

            # ---- global attention pipeline ----
            po = None
            ptv0 = pt0.rearrange("p (s t) -> p s t", s=2)
            for g, (i, jj) in enumerate(PAIRS):
                if jj == 0:
                    po = accp.tile([P, QT], f32, tag="po", name=f"po{i}")
                if g == 0:
                    # odd half of the split first pair
                    w1 = width(0, 1)
                    nc.scalar.activation(ptv0[:, 1, 0:w1],
                                         ps0.rearrange("p (s t) -> p s t", s=2)[:, 1, 0:w1],
                                         EXP, scale=ACT_SCALE)
                    pt = pt0
                else:
                    pt = exp_pair(i, jj, sp_live.pop(g))
                masks(i, jj, pt)
                if g + 2 < len(PAIRS):
                    ni = PAIRS[g + 2][0]
                    if ni >= 2:
                        while pend[ni]:
                            pend[ni].pop(0)()
                    sp_live[g + 2] = s_pair(*PAIRS[g + 2])
                pv_pair(i, jj, pt, po)
                pop_proj(4)
                if jj == 2 * (i + 1) - 1:
                    finish_block(i, po)

    nc.compile()
    return nc


_NC_CACHE = None


def _get_nc():
    global _NC_CACHE
    if _NC_CACHE is None:
        _NC_CACHE = build_nc()
    return _NC_CACHE


def run(in_maps, trace=False, **kw):
    nc = _get_nc()
    return run_bass_kernel_spmd(nc, in_maps, core_ids=list(range(B)),
                                trace=trace, **kw)


def _pack_weights(Wq, Wk, Wv):
    wb = np.zeros((P, WCOLS), dtype=np.float32)
    scale = np.float32(ASCHR / np.sqrt(H))
    for c in range(NCH):
        rows = slice(c * P, (c + 1) * P)
        wb[:, WKV0 + c * P : WKV0 + c * P + H] = Wk[rows, :]
        wb[:, WKV0 + c * P + H : WKV0 + (c + 1) * P] = Wv[rows, :]
        wb[:, WQK0 + c * P : WQK0 + c * P + H] = Wq[rows, :] * scale
        wb[:, WQK0 + c * P + H : WQK0 + (c + 1) * P] = Wk[rows, :]
    wb[:, WID0 : WID0 + P] = np.eye(P, dtype=np.float32)
    wb[0:H, WSH0 + H : WSH0 + P] = np.eye(H, dtype=np.float32)
    return wb.astype(BF16NP)


def make_in_maps(x, Wq, Wk, Wv):
    x = np.asarray(x, dtype=np.float32)
    Wq = np.asarray(Wq, dtype=np.float32)
    Wk = np.asarray(Wk, dtype=np.float32)
    Wv = np.asarray(Wv, dtype=np.float32)
    wb = _pack_weights(Wq, Wk, Wv)
    ins = []
    for b in range(B):
        A = x[b].reshape(NQ, QT, NCH, P).transpose(3, 0, 2, 1)
        ins.append({
            "X": np.ascontiguousarray(A).astype(BF16NP).reshape(P, NQ * NCH * QT),
            "WB": wb,
        })
    return ins


def kernel(x, Wq, Wk, Wv):
    res = run(make_in_maps(x, Wq, Wk, Wv))
    return np.stack([res.results[b]["out"] for b in range(B)], axis=0)
